# revision 1
# baseline (speedup 1.0000x reference)
"""Trainium2 Bass kernel for CIN layer:
    out[b,c,d] = sum_{h,m} W[c, h*M+m] * xk[b,h,d] * x0[b,m,d] + bias[c]

Shapes (hardcoded): x0 [512,40,64] f32, xk [512,128,64] f32,
W [128,5120] f32, b [128] f32 -> out [512,128,64] f32.

Strategy: data-parallel over batch B across 8 cores (64 batches/core).
Per core, columns are the 64*64=4096 (b,d) pairs.  The 5120-long (h,m)
contraction is split into 40 chunks of 128 rows with a mixed-radix
partition layout: chunk (g, j) covers m in the 8-wide group g (5
groups) x h in the 16-wide block j (8 blocks); partition p holds
(m = 8g + p//16, h = 16j + p%16).  Then per chunk
  outer[p, col] = xkrep_j[p, col] * x0bc_g[p, col]   (elementwise bf16)
  psum[bank]   += w3[k][p,c].T @ outer[:, bank*512:] (PE, 40-chunk accum)
xkrep_j / x0bc_g are replicated host-side (pure layout, no math).

Engine balance: the 21M-elem/core outer-product stream exceeds DVE
capacity (bf16 tensor_tensor caps at 2 elem/lane/cycle, ~75us), so
~1 chunk per 5-chunk group is produced on GpSimd (~4.2us per half-
chunk there, 0.42x roofline).  GpSimd chunks are consumed 1-2 groups
after production so the slow engine is never on the PE critical path.

Column passes: A=[0,1024) into PSUM banks 0-1, B=[1024,2048) banks
2-3, C=[2048,4096) banks 4-7.  The narrow bootstrap passes let compute
start ~4us in, while the prologue DMA stream (14.9MB, ~35us serialized
on the DMA engines) is still arriving; quarter-granularity loads are
ordered by first use.  Dep-free scratch matmuls pad the PE stream
through pass A so momentary supply stalls cannot idle the PE (an idle
gap drops the PE clock from 2.4 to 1.2 GHz for the next ~3us).  Pass-C
eviction (ScalarE bias-add) is per-bank with single-bank stores so the
post-last-matmul tail is one evict + one 728ns store.

DMA-descriptor shaping: W is pre-transposed to [128, 40*128] (10KB
contiguous per partition) and the output DRAM tensor is c-major
[C, BC, D] (2KB contiguous runs); descriptors under 512B pay a 2x
DMA-time penalty.  bias rides early so evictions never FIFO-wait on
the whole prologue.  The reps>1 build (used for steady-state timing)
drops the bootstrap: two half-width passes with the same GpSimd
interleave.
"""

import numpy as np
import ml_dtypes

B, M, H, D, C = 512, 40, 128, 64, 128
N_CORES = 8
BC = B // N_CORES          # 64 batches per core
COLS = BC * D              # 4096 (b,d) columns per core
NG = 8                     # PSUM groups
GW = COLS // NG            # 512 columns per group
MG = 8                     # m-values per chunk group
NMG = M // MG              # 5 m-groups
HB = 128 // MG             # 16 h-values per block
NHB = H // HB              # 8 h-blocks
NCHUNK = NMG * NHB         # 40 contraction chunks

_cache = {}


def _build(reps=1, unroll=False):
    import contextlib

    import concourse.bacc as bacc
    import concourse.mybir as mybir
    from concourse.tile import TileContext

    f32 = mybir.dt.float32
    bf16 = mybir.dt.bfloat16

    nc = bacc.Bacc("TRN2", debug=False, num_devices=N_CORES)

    xkr_d = nc.dram_tensor("xkrep_in", [NHB, 128, COLS], bf16, kind="ExternalInput")
    x0b_d = nc.dram_tensor("x0bc_in", [NMG, 128, COLS], bf16, kind="ExternalInput")
    # pre-transposed: partition-major, 10KB contiguous per partition row
    w3_d = nc.dram_tensor("w3_in", [128, NCHUNK * C], bf16, kind="ExternalInput")
    bias_d = nc.dram_tensor("bias_in", [C, 1], f32, kind="ExternalInput")
    # c-major so each output descriptor is a contiguous 2KB (b,d) run
    out_d = nc.dram_tensor("out", [C, BC, D], f32, kind="ExternalOutput")

    GK = 5
    NGRP = NCHUNK // GK        # 8 groups per phase
    ND_SLOT = 9
    NG_SLOT = 4
    HCOL = COLS // 2
    QCOL = COLS // 4

    with TileContext(nc) as tc:
        with (
            tc.tile_pool(name="const", bufs=1) as cpool,
            tc.tile_pool(name="work", bufs=6) as wpool,
            tc.tile_pool(name="outp", bufs=1) as opool,
            tc.tile_pool(name="psum", bufs=1, space="PSUM") as ppool,
        ):
            # ---- SBUF constant tiles ----
            w3_sb = cpool.tile([128, NCHUNK * C], bf16)
            bias_sb = cpool.tile([128, 1], f32)
            xkall = cpool.tile([128, NHB * COLS], bf16, name="xkall")
            xkreps = [xkall[:, i * COLS:(i + 1) * COLS] for i in range(NHB)]
            xk3 = xkall.rearrange("p (j c) -> p j c", c=COLS)
            x0bcs = [
                cpool.tile([128, COLS], bf16, name=f"x0b{i}", tag=f"x0b{i}")
                for i in range(NMG)
            ]

            # ---- prologue DMA, first-use order ----
            # Group-0 operand tiles at quarter-column granularity (with the
            # w3 chunk-0..9 slice third) so the first TT/MM chain starts
            # ~2.5us in; then everything else in first-use order.
            def load_tile(kind, i, c0, c1):
                t = xkreps[i] if kind == "x" else x0bcs[i]
                src = (xkr_d if kind == "x" else x0b_d).ap()[i]
                nc.sync.dma_start(out=t[:, c0:c1], in_=src[:, c0:c1])

            use_order = [("x", 0), ("0", 0), ("x", 1), ("x", 2), ("x", 3),
                         ("x", 4), ("x", 5), ("x", 6), ("0", 1), ("x", 7),
                         ("0", 2), ("0", 3), ("0", 4)]
            # Two parallel DMA streams: the sync queue carries the
            # bootstrap-critical pass-A/B quarters (w3 thirds interleaved,
            # first-use order); the otherwise-idle ScalarE queue carries the
            # pass-C halves concurrently so pass C's operands are resident
            # long before it starts.  bias is tiny and only needed by the
            # first eviction (~25us in), so it rides third.
            for n, (kind, i) in enumerate(use_order):
                load_tile(kind, i, 0, QCOL)
                if n == 1:
                    nc.sync.dma_start(
                        out=w3_sb[:, : 10 * C], in_=w3_d.ap()[:, : 10 * C]
                    )
                elif n == 12:
                    # needed first by evict0 (~25us); issuing it here keeps
                    # the HWDGE slot out of the bootstrap-critical stream
                    nc.sync.dma_start(out=bias_sb, in_=bias_d.ap())
                elif n == 4:
                    nc.sync.dma_start(
                        out=w3_sb[:, 10 * C: 20 * C],
                        in_=w3_d.ap()[:, 10 * C: 20 * C],
                    )
                elif n == 6:
                    nc.sync.dma_start(
                        out=w3_sb[:, 20 * C:], in_=w3_d.ap()[:, 20 * C:]
                    )
            for kind, i in use_order:
                load_tile(kind, i, QCOL, HCOL)
            for kind, i in use_order:
                load_tile(kind, i, HCOL, COLS)

            loop_ctx = (
                tc.For_i(
                    0, reps, 1,
                    hint_engines=(mybir.EngineType.PE,),
                    staggered_reset=True,
                )
                if reps > 1 and not unroll
                else contextlib.nullcontext()
            )
            n_unroll = reps if unroll else 1
            with loop_ctx:
                psums = []
                for q in range(NG):
                    ps = ppool.tile([128, GW], f32, name=f"ps{q}", tag=f"ps{q}")
                    psums.append(ps)

                if reps == 1:
                    # Warm the PE (HAM clock-gate needs ~3.4us of sustained
                    # activity to reach 2.4 GHz) with dummy matmuls while
                    # the first operand DMAs are in flight.  start=True on
                    # each real first-accumulation MM discards the garbage.
                    scratch = cpool.tile([128, GW], bf16)
                    nc.gpsimd.memset(scratch, 0.0)
                    for _ in range(12):
                        nc.tensor.matmul(
                            psums[0],
                            lhsT=scratch[:, :128],
                            rhs=scratch,
                            start=True,
                            stop=True,
                        )

                for _it in range(n_unroll):
                    # ---- main loop: three column passes over 8 chunk groups --
                    # Pass A covers cols [0,1024) into PSUM banks 0-1, pass B
                    # [1024,2048) into banks 2-3, pass C [2048,4096) into banks
                    # 4-7.  The narrow bootstrap passes keep the PE fed while
                    # the prologue DMA stream is still arriving; pass C uses
                    # full half-width ops once everything is resident.
                    # GpSimd-produced chunks are consumed `lag` groups after
                    # production so the slow engine never gates the PE; pass
                    # A/B picks respect prologue DMA arrival times.
                    # sched = {chunk: cons_group}
                    gp_a = {21: 5, 31: 7}
                    gp_b = {1: 3, 6: 4, 11: 5, 21: 7}
                    gp_c = {9: 3, 14: 5, 24: 7}
                    # real-HW GpSimd tensor_mul runs far below the cost
                    # model's 0.42x-roofline figure (measured ~0.15x), so
                    # the steady loop keeps the outer-product stream
                    # entirely on the DVE; the single-shot build uses a
                    # few GpSimd chunks only to smooth the bootstrap
                    gp_s = {}
                    if reps == 1:
                        # single-shot: narrow bootstrap passes overlap the
                        # prologue DMA stream
                        passes = [
                            ("A", 0, QCOL, [0, 1], gp_a, 9, 4, False, 1),
                            ("B", QCOL, 2 * QCOL, [2, 3], gp_b, 9, 4,
                             False, 1),
                            ("C", 2 * QCOL, 4 * QCOL, [4, 5, 6, 7], gp_c,
                             8, 4, False, 1),
                        ]
                    else:
                        # steady-state loop: everything is resident; two
                        # half-width phases minimize per-iteration op count
                        passes = [
                            ("A", 0, HCOL, [0, 1, 2, 3], gp_s, 8, 1,
                             False, 1),
                            ("C", HCOL, 2 * HCOL, [4, 5, 6, 7], gp_s, 8, 1,
                             False, 1),
                        ]
                    bpg = BC // NG
                    for (pname, c0, c1, banks, sched, n_od, n_og,
                         pair_c, n_odp) in passes:
                        width = c1 - c0
                        nq = width // GW
                        ndve = 0
                        ngp = 0
                        pending = {}    # cons_group -> [(k, tile)]
                        for gi in range(NGRP):
                            k0 = gi * GK
                            last_grp = gi == NGRP - 1
                            mm_list = list(pending.pop(gi, []))
                            dve_ks = []
                            for k in range(k0, k0 + GK):
                                g, j = divmod(k, NHB)
                                if k in sched:
                                    outer = wpool.tile(
                                        [128, width], bf16,
                                        name=f"outg{pname}_{k}",
                                        tag=f"og{width}_{ngp % n_og}", bufs=1,
                                    )
                                    ngp += 1
                                    nc.gpsimd.tensor_mul(
                                        outer,
                                        xkreps[j][:, c0:c1],
                                        x0bcs[g][:, c0:c1],
                                    )
                                    pending.setdefault(sched[k], []).append(
                                        (k, outer, 0)
                                    )
                                else:
                                    dve_ks.append(k)
                            # pair adjacent chunks sharing an x0bc tile into one
                            # double-width TT (one DVE dispatch, same elements):
                            # in0 spans two adjacent xkrep slices of the merged
                            # xkall tile, in1 broadcasts the shared x0bc slice
                            units = []
                            i = 0
                            while i < len(dve_ks):
                                k = dve_ks[i]
                                if (pair_c and i + 1 < len(dve_ks)
                                        and dve_ks[i + 1] == k + 1
                                        and k % NHB != NHB - 1):
                                    units.append((k, 2))
                                    i += 2
                                else:
                                    units.append((k, 1))
                                    i += 1
                            for k, nun in units:
                                g, j = divmod(k, NHB)
                                if nun == 2:
                                    outer = wpool.tile(
                                        [128, 2 * width], bf16,
                                        name=f"outp{pname}_{k}",
                                        tag=f"odp{width}_{ndve % n_odp}", bufs=1,
                                    )
                                    ndve += 1
                                    nc.vector.tensor_mul(
                                        outer.rearrange(
                                            "p (t c) -> p t c", t=2
                                        ),
                                        xk3[:, j:j + 2, c0:c1],
                                        x0bcs[g][:, c0:c1]
                                        .unsqueeze(1)
                                        .broadcast_to([128, 2, width]),
                                    )
                                    mm_list.append((k, outer, 0))
                                    mm_list.append((k + 1, outer, width))
                                else:
                                    outer = wpool.tile(
                                        [128, width], bf16,
                                        name=f"outd{pname}_{k}",
                                        tag=f"od{width}_{ndve % n_od}", bufs=1,
                                    )
                                    ndve += 1
                                    nc.vector.tensor_mul(
                                        outer,
                                        xkreps[j][:, c0:c1],
                                        x0bcs[g][:, c0:c1],
                                    )
                                    mm_list.append((k, outer, 0))
                            for qi, qb in enumerate(banks):
                                for n, (k, outer, off) in enumerate(mm_list):
                                    nc.tensor.matmul(
                                        psums[qb],
                                        lhsT=w3_sb[:, k * C:(k + 1) * C],
                                        rhs=outer[:, off + qi * GW:
                                                  off + (qi + 1) * GW],
                                        start=(k == 0),
                                        stop=(last_grp and n == len(mm_list) - 1),
                                    )
                            if reps == 1 and pname == "A" and gi < 6:
                                # bootstrap filler: the prologue DMA stream
                                # can momentarily starve the PE here; dep-
                                # free scratch matmuls into a not-yet-active
                                # bank absorb the stall instead of letting
                                # the PE idle (an idle gap resets the clock
                                # ramp, costing ~2x on the next ~3us of MMs)
                                for _ in range(4):
                                    nc.tensor.matmul(
                                        psums[7],
                                        lhsT=scratch[:, :128],
                                        rhs=scratch,
                                        start=True,
                                        stop=True,
                                    )
                        assert not pending
                        # bias add into a shared SBUF tile; contiguous multi-bank
                        # stores (>=2KB per-partition descriptors).  The final
                        # pass stores 3+1 banks so the post-last-matmul chain is
                        # only one bank's evict + a 512-col store.
                        # per-bank eviction tiles on two rotating buffers:
                        # a shared per-pass tile would serialize the
                        # evict/store chain through false whole-tile deps.
                        # Single-bank c-major stores (2KB descriptors) keep
                        # the post-last-matmul tail to one evict + one
                        # 728ns store.
                        for qi, qb in enumerate(banks):
                            out_sb = opool.tile(
                                [128, GW], f32, name=f"osb{pname}{qb}",
                                tag=f"osb{qi}",
                            )
                            nc.scalar.activation(
                                out_sb,
                                psums[qb],
                                mybir.ActivationFunctionType.Identity,
                                bias=bias_sb[:, 0:1],
                                scale=1.0,
                            )
                            nc.sync.dma_start(
                                out=out_d.ap()[:, qb * bpg:(qb + 1) * bpg, :],
                                in_=out_sb,
                            )

    nc.compile()
    return nc


def _prep_host(x0, xk, W, b):
    """Host-side layout prep (no arithmetic): shard, transpose, replicate."""
    part = np.arange(128)
    hh = (part % HB)[None, :] + HB * np.arange(NHB)[:, None]   # [NHB, 128]
    mm = (part // HB)[None, :] + MG * np.arange(NMG)[:, None]  # [NMG, 128]

    Wr = W.reshape(C, H, M)
    w3 = np.empty((128, NCHUNK, C), ml_dtypes.bfloat16)
    for g in range(NMG):
        for j in range(NHB):
            w3[:, g * NHB + j, :] = Wr[:, hh[j], mm[g]].T.astype(
                ml_dtypes.bfloat16
            )
    w3 = np.ascontiguousarray(w3.reshape(128, NCHUNK * C))
    bias = np.ascontiguousarray(b.reshape(C, 1)).astype(np.float32)

    in_maps = []
    for k in range(N_CORES):
        x0s = x0[k * BC:(k + 1) * BC]            # [BC, M, D]
        xks = xk[k * BC:(k + 1) * BC]            # [BC, H, D]
        xk2 = (
            np.ascontiguousarray(xks.transpose(1, 0, 2))
            .reshape(H, COLS)
            .astype(ml_dtypes.bfloat16)
        )
        x02 = (
            np.ascontiguousarray(x0s.transpose(1, 0, 2))
            .reshape(M, COLS)
            .astype(ml_dtypes.bfloat16)
        )
        in_maps.append(
            {
                "xkrep_in": np.ascontiguousarray(xk2[hh]),
                "x0bc_in": np.ascontiguousarray(x02[mm]),
                "w3_in": w3,
                "bias_in": bias,
            }
        )
    return in_maps


def _run(in_maps, **kwargs):
    from concourse import bass_utils

    if "nc" not in _cache:
        _cache["nc"] = _build()
    return bass_utils.run_bass_kernel_spmd(
        _cache["nc"], in_maps, core_ids=list(range(N_CORES)), **kwargs
    )


def kernel(x0, xk, W, b, _bench=[None]):
    x0 = np.asarray(x0, dtype=np.float32)
    xk = np.asarray(xk, dtype=np.float32)
    W = np.asarray(W, dtype=np.float32)
    b = np.asarray(b, dtype=np.float32)
    in_maps = _prep_host(x0, xk, W, b)
    res = _run(in_maps)
    _bench[0] = res
    # per-core out is c-major [C, BC, D]; restore [BC, C, D] and stack cores
    out = np.concatenate(
        [np.transpose(r["out"], (1, 0, 2)) for r in res.results], axis=0
    )
    return np.ascontiguousarray(out, dtype=np.float32)



# revision 4
# speedup vs baseline: 14.3434x; 14.3434x over previous
"""Trainium2 Bass kernel for CIN layer:
    out[b,c,d] = sum_{h,m} W[c, h*M+m] * xk[b,h,d] * x0[b,m,d] + bias[c]

Shapes (hardcoded): x0 [512,40,64] f32, xk [512,128,64] f32,
W [128,5120] f32, b [128] f32 -> out [512,128,64] f32.

Strategy: data-parallel over batch B across 8 cores (64 batches/core).
Per core, columns are the 64*64=4096 (b,d) pairs.  The 5120-long (h,m)
contraction is split into 40 chunks of 128 rows with a mixed-radix
partition layout: chunk k=(g*8+j) covers m in the 8-wide group g (5
groups) x h in the 16-wide block j (8 blocks); partition p holds
(m = 8g + p//16, h = 16j + p%16).  Then per chunk
  outer[p, col] = xkrep_j[p, col] * x0bc_g[p, col]   (elementwise bf16)
  psum[bank]   += w3[k][p,c].T @ outer[:, bank*512:] (PE, 40-chunk accum)
xkrep_j / x0bc_g are replicated host-side (pure layout, no math).

Engine balance (HW-measured rates): the 21M-elem/core outer-product
stream costs 85.5us if the DVE makes all of it (TensorTensor bf16 runs
in 2x mode: 2 elem/lane/cycle at 0.96 GHz = 245.8 Gel/s), but the PE
only needs 68.3us for its 320 matmuls, so the DVE would be the
bottleneck.  GpSimd (Pool) TensorTensor measures 62.8 Gel/s, so the
split is 32 chunks/pass on DVE (as 16 paired double-width ops sharing
an x0 slice) + 8 chunks/pass on Pool: DVE ~36us/pass, Pool ~35us/pass,
PE ~34us/pass -- all three engines land together at the PE roofline.
Pool chunks are emitted first per group and buffered (3 rotating bufs)
so the slow engine runs ~2 groups ahead and never gates the PE.

Column passes: two half-width passes, A=[0,2048) into PSUM banks 0-3,
B=[2048,4096) into banks 4-7.  The prologue DMA stream (14.9MB, ~43us
at 332 GB/s) is issued at half-column granularity in first-use order on
the sync queue, so pass A's operands land in the first ~7us while pass
A itself runs ~37us; pass B's halves arrive long before pass B starts.
Dep-free scratch matmuls pad the PE stream through the boot window
(an idle gap drops the PE clock from 2.4 to 1.2 GHz for the next ~3us).
Pass eviction (ScalarE bias-add) is per-bank; stores ride the ScalarE
DGE queue so they never FIFO-wait behind the load stream.

DMA-descriptor shaping: W is pre-transposed to [128, 40*128] (10KB
contiguous per partition) and the output DRAM tensor is c-major
[C, BC, D] (2KB contiguous runs); descriptors under 512B pay a 2x
DMA-time penalty.  The reps>1 build (used for steady-state timing)
is the same two-pass body inside a For_i hardware loop.
"""

import numpy as np
import ml_dtypes

B, M, H, D, C = 512, 40, 128, 64, 128
N_CORES = 8
BC = B // N_CORES          # 64 batches per core
COLS = BC * D              # 4096 (b,d) columns per core
NG = 8                     # PSUM banks
GW = COLS // NG            # 512 columns per bank
MG = 8                     # m-values per chunk group
NMG = M // MG              # 5 m-groups
HB = 128 // MG             # 16 h-values per block
NHB = H // HB              # 8 h-blocks
NCHUNK = NMG * NHB         # 40 contraction chunks

_cache = {}


def _build(reps=1, n_warm=14, n_fill=3):
    import contextlib

    import concourse.bacc as bacc
    import concourse.mybir as mybir
    from concourse.tile import TileContext

    f32 = mybir.dt.float32
    bf16 = mybir.dt.bfloat16

    nc = bacc.Bacc("TRN2", debug=False, num_devices=N_CORES)

    xkr_d = nc.dram_tensor("xkrep_in", [NHB, 128, COLS], bf16, kind="ExternalInput")
    x0b_d = nc.dram_tensor("x0bc_in", [NMG, 128, COLS], bf16, kind="ExternalInput")
    # pre-transposed: partition-major, 10KB contiguous per partition row
    w3_d = nc.dram_tensor("w3_in", [128, NCHUNK * C], bf16, kind="ExternalInput")
    bias_d = nc.dram_tensor("bias_in", [C, 1], f32, kind="ExternalInput")
    # c-major so each output descriptor is a contiguous 2KB (b,d) run
    out_d = nc.dram_tensor("out", [C, BC, D], f32, kind="ExternalOutput")

    GK = 5
    NGRP = NCHUNK // GK        # 8 groups per pass
    HCOL = COLS // 2           # 2048
    bpg = BC // NG             # 8 batches per bank

    # Per 5-chunk group: one chunk to Pool, remaining four as two adjacent
    # (same-g, j/j+1) DVE pairs.  Chosen so every group pairs cleanly.
    POOL_CHUNKS = (0, 7, 10, 15, 24, 25, 34, 35)
    GROUPS = []
    for gi in range(NGRP):
        ks = list(range(gi * GK, gi * GK + GK))
        pk = [k for k in ks if k in POOL_CHUNKS]
        assert len(pk) == 1
        rest = [k for k in ks if k != pk[0]]
        pairs = [(rest[0], rest[1]), (rest[2], rest[3])]
        for a, b in pairs:
            assert b == a + 1 and a % NHB != NHB - 1 and a // NHB == b // NHB
        GROUPS.append((pk[0], pairs))

    N_DVE_BUF = 6
    N_POOL_BUF = 3

    with TileContext(nc) as tc:
        with (
            tc.tile_pool(name="const", bufs=1) as cpool,
            tc.tile_pool(name="work", bufs=1) as wpool,
            tc.tile_pool(name="outp", bufs=1) as opool,
            tc.tile_pool(name="psum", bufs=1, space="PSUM") as ppool,
        ):
            # ---- SBUF constant tiles ----
            w3_sb = cpool.tile([128, NCHUNK * C], bf16)
            bias_sb = cpool.tile([128, 1], f32)
            xkall = cpool.tile([128, NHB * COLS], bf16, name="xkall")
            xkreps = [xkall[:, i * COLS:(i + 1) * COLS] for i in range(NHB)]
            xk3 = xkall.rearrange("p (j c) -> p j c", c=COLS)
            x0bcs = [
                cpool.tile([128, COLS], bf16, name=f"x0b{i}", tag=f"x0b{i}")
                for i in range(NMG)
            ]

            # ---- prologue DMA: half-column granularity, first-use order ---
            # Single (sync-queue) stream at full DMA bandwidth.  w3 rides in
            # three slices ordered by the chunk ranges that consume them;
            # bias is only needed by the first eviction (~40us in).
            def ld(kind, i, c0, c1):
                t = xkreps[i] if kind == "x" else x0bcs[i]
                src = (xkr_d if kind == "x" else x0b_d).ap()[i]
                nc.sync.dma_start(out=t[:, c0:c1], in_=src[:, c0:c1])

            use_order = [("0", 0), ("x", 1), ("x", 2), ("w", 0), ("x", 0),
                         ("x", 3), ("x", 4), ("0", 1), ("x", 5), ("x", 6),
                         ("w", 1), ("x", 7), ("0", 2), ("w", 2), ("0", 3),
                         ("0", 4)]
            w_slices = [(0, 10 * C), (10 * C, 25 * C), (25 * C, NCHUNK * C)]
            for half, (c0, c1) in enumerate([(0, HCOL), (HCOL, COLS)]):
                for kind, i in use_order:
                    if kind == "w":
                        if half == 0:
                            s0, s1 = w_slices[i]
                            nc.sync.dma_start(
                                out=w3_sb[:, s0:s1], in_=w3_d.ap()[:, s0:s1]
                            )
                    else:
                        ld(kind, i, c0, c1)
                if half == 0:
                    nc.sync.dma_start(out=bias_sb, in_=bias_d.ap())

            loop_ctx = (
                tc.For_i(
                    0, reps, 1,
                    hint_engines=(mybir.EngineType.PE,),
                    staggered_reset=True,
                )
                if reps > 1
                else contextlib.nullcontext()
            )
            with loop_ctx:
                psums = [
                    ppool.tile([128, GW], f32, name=f"ps{q}", tag=f"ps{q}")
                    for q in range(NG)
                ]

                if reps == 1:
                    # Warm the PE (clock-gate needs ~3.4us of sustained
                    # activity to reach 2.4 GHz) with dummy matmuls while
                    # the first operand DMAs are in flight.  Pass A uses
                    # banks 0-3, so bank 7 absorbs the garbage; pass B's
                    # first real MM into bank 7 resets it via start=True.
                    scratch = cpool.tile([128, GW], bf16)
                    nc.gpsimd.memset(scratch, 0.0)
                    for _ in range(n_warm):
                        nc.tensor.matmul(
                            psums[7], lhsT=scratch[:, :128], rhs=scratch,
                            start=True, stop=True,
                        )

                ndve = 0
                npool = 0
                passes = [(0, HCOL, (0, 1, 2, 3)), (HCOL, COLS, (4, 5, 6, 7))]
                for pi, (c0, c1, banks) in enumerate(passes):
                    width = c1 - c0
                    for gi, (pool_k, pairs) in enumerate(GROUPS):
                        entries = []
                        # Pool chunk first so the slow engine's stream is
                        # maximally early; 3 rotating bufs let it run ahead.
                        g, j = divmod(pool_k, NHB)
                        po = wpool.tile(
                            [128, width], bf16, name=f"po{pi}_{gi}",
                            tag=f"po{npool % N_POOL_BUF}", bufs=1,
                        )
                        npool += 1
                        nc.gpsimd.tensor_mul(
                            po, xkreps[j][:, c0:c1], x0bcs[g][:, c0:c1]
                        )
                        for ka, kb in pairs:
                            g2, j2 = divmod(ka, NHB)
                            t = wpool.tile(
                                [128, 2 * width], bf16,
                                name=f"od{pi}_{gi}_{ka}",
                                tag=f"od{ndve % N_DVE_BUF}", bufs=1,
                            )
                            ndve += 1
                            nc.vector.tensor_mul(
                                t.rearrange("p (u c) -> p u c", u=2),
                                xk3[:, j2:j2 + 2, c0:c1],
                                x0bcs[g2][:, c0:c1]
                                .unsqueeze(1)
                                .broadcast_to([128, 2, width]),
                            )
                            entries.append((ka, t, 0))
                            entries.append((kb, t, width))
                        # Pool chunk consumed last: it was produced with ~1
                        # group of lag.  Chunk-major MM order (all banks per
                        # chunk) so the PE consumes each chunk the moment it
                        # is produced instead of stalling a whole bank sweep
                        # on the last chunk of the group; also reuses the
                        # stationary w3 chunk across the 4 bank MMs.
                        entries.append((pool_k, po, 0))
                        for n, (k, t, off) in enumerate(entries):
                            for qi, qb in enumerate(banks):
                                nc.tensor.matmul(
                                    psums[qb],
                                    lhsT=w3_sb[:, k * C:(k + 1) * C],
                                    rhs=t[:, off + qi * GW:
                                          off + (qi + 1) * GW],
                                    start=(gi == 0 and n == 0),
                                    stop=(gi == NGRP - 1
                                          and n == len(entries) - 1),
                                )
                        if reps == 1 and pi == 0 and gi < 6:
                            # bootstrap filler: the prologue DMA stream can
                            # momentarily starve the PE here; dep-free
                            # scratch matmuls into the not-yet-active bank 7
                            # absorb the stall (an idle gap resets the clock
                            # ramp, costing ~2x on the next ~3us of MMs)
                            for _ in range(n_fill):
                                nc.tensor.matmul(
                                    psums[7], lhsT=scratch[:, :128],
                                    rhs=scratch, start=True, stop=True,
                                )
                    # bias-add eviction per bank on ScalarE; stores ride the
                    # ScalarE DGE queue so they never queue behind loads.
                    for qi, qb in enumerate(banks):
                        out_sb = opool.tile(
                            [128, GW], f32, name=f"osb{pi}{qb}",
                            tag=f"osb{qi}",
                        )
                        nc.scalar.activation(
                            out_sb,
                            psums[qb],
                            mybir.ActivationFunctionType.Identity,
                            bias=bias_sb[:, 0:1],
                            scale=1.0,
                        )
                        nc.scalar.dma_start(
                            out=out_d.ap()[:, qb * bpg:(qb + 1) * bpg, :],
                            in_=out_sb,
                        )

    nc.compile()
    return nc


def _prep_host(x0, xk, W, b):
    """Host-side layout prep (no arithmetic): shard, transpose, replicate."""
    part = np.arange(128)
    hh = (part % HB)[None, :] + HB * np.arange(NHB)[:, None]   # [NHB, 128]
    mm = (part // HB)[None, :] + MG * np.arange(NMG)[:, None]  # [NMG, 128]

    Wr = W.reshape(C, H, M)
    w3 = np.empty((128, NCHUNK, C), ml_dtypes.bfloat16)
    for g in range(NMG):
        for j in range(NHB):
            w3[:, g * NHB + j, :] = Wr[:, hh[j], mm[g]].T.astype(
                ml_dtypes.bfloat16
            )
    w3 = np.ascontiguousarray(w3.reshape(128, NCHUNK * C))
    bias = np.ascontiguousarray(b.reshape(C, 1)).astype(np.float32)

    in_maps = []
    for k in range(N_CORES):
        x0s = x0[k * BC:(k + 1) * BC]            # [BC, M, D]
        xks = xk[k * BC:(k + 1) * BC]            # [BC, H, D]
        xk2 = (
            np.ascontiguousarray(xks.transpose(1, 0, 2))
            .reshape(H, COLS)
            .astype(ml_dtypes.bfloat16)
        )
        x02 = (
            np.ascontiguousarray(x0s.transpose(1, 0, 2))
            .reshape(M, COLS)
            .astype(ml_dtypes.bfloat16)
        )
        in_maps.append(
            {
                "xkrep_in": np.ascontiguousarray(xk2[hh]),
                "x0bc_in": np.ascontiguousarray(x02[mm]),
                "w3_in": w3,
                "bias_in": bias,
            }
        )
    return in_maps


def _run(in_maps, **kwargs):
    from concourse import bass_utils

    if "nc" not in _cache:
        _cache["nc"] = _build()
    return bass_utils.run_bass_kernel_spmd(
        _cache["nc"], in_maps, core_ids=list(range(N_CORES)), **kwargs
    )


def kernel(x0, xk, W, b, _bench=[None]):
    x0 = np.asarray(x0, dtype=np.float32)
    xk = np.asarray(xk, dtype=np.float32)
    W = np.asarray(W, dtype=np.float32)
    b = np.asarray(b, dtype=np.float32)
    in_maps = _prep_host(x0, xk, W, b)
    res = _run(in_maps)
    _bench[0] = res
    # per-core out is c-major [C, BC, D]; restore [BC, C, D] and stack cores
    out = np.concatenate(
        [np.transpose(r["out"], (1, 0, 2)) for r in res.results], axis=0
    )
    return np.ascontiguousarray(out, dtype=np.float32)


# revision 36
# speedup vs baseline: 19.6184x; 1.3678x over previous
"""Trainium2 Bass kernel for CIN layer:
    out[b,c,d] = sum_{h,m} W[c, h*M+m] * xk[b,h,d] * x0[b,m,d] + bias[c]

Shapes (hardcoded): x0 [512,40,64] f32, xk [512,128,64] f32,
W [128,5120] f32, b [128] f32 -> out [512,128,64] f32.

Strategy: data-parallel over batch B across 8 cores (64 batches/core).
Per core, columns are the 64*64=4096 (b,d) pairs.  The 5120-long (h,m)
contraction is split into 40 chunks of 128 rows with a mixed-radix
partition layout: chunk k=(g*8+j) covers m in the 8-wide group g (5
groups) x h in the 16-wide block j (8 blocks); partition p holds
(m = 8g + p//16, h = 16j + p%16).  Then per chunk
  outer[p, col] = xkrep_j[p, col] * x0bc_g[p, col]   (elementwise bf16)
  psum[bank]   += w3[k][p,c].T @ outer[:, bank*512:] (PE, 40-chunk accum)
xkrep_j / x0bc_g are replicated host-side (pure layout, no math).

Engine balance (HW-measured): the 21M-elem/core outer-product stream
is produced ENTIRELY on the DVE (TensorTensor bf16 2x mode: 2 elem/
lane/cycle at 0.96 GHz = 245.8 Gel/s -> ~89us/iter), adjacent same-g
chunk pairs fused into one double-width op with the shared x0 slice
broadcast along the middle dim (saves the per-op ramp overhead).  The
PE only needs 68.3us for its 320 matmuls, so it trails the DVE.
IMPORTANT NEGATIVE RESULT: offloading chunks to GpSimd/Pool (62.8
Gel/s solo) is a large net LOSS in situ -- even dep-free dummy Pool
TensorTensors alongside the DVE+PE stream blow per-iter time from
~94us to ~159us (SBUF bandwidth contention).  Keep Pool idle.

Column passes: two half-width passes, A=[0,2048) into PSUM banks 0-3,
B=[2048,4096) into banks 4-7.  MM emission is chunk-major (all 4 bank
MMs per chunk back-to-back) so the PE consumes each outer tile the
moment the DVE finishes it and the tile's buffer frees quickly --
bank-major sweeps hold buffers a whole group and stall the DVE on
tag-rotation WAR (~+9us/pass, measured).  The prologue DMA stream
(14.9MB, ~45us at 332 GB/s) is issued at half-column granularity in
first-use order on the sync queue, so pass A's operands land in the
first ~7us while pass A runs ~45us.  Dep-free scratch matmuls pad the
PE stream through the boot window (an idle gap drops the PE clock from
2.4 to 1.2 GHz for the next ~3us).  Pass eviction (ScalarE bias-add)
is per-bank; stores ride the ScalarE DGE queue so they never FIFO-wait
behind the load stream.

DMA-descriptor shaping: W is pre-transposed to [128, 40*128] (10KB
contiguous per partition) and the output DRAM tensor is c-major
[C, BC, D] (2KB contiguous runs); descriptors under 512B pay a 2x
DMA-time penalty.  The reps>1 build (used for steady-state timing)
is the same two-pass body inside a For_i hardware loop.
"""

import numpy as np
import ml_dtypes

B, M, H, D, C = 512, 40, 128, 64, 128
N_CORES = 8
BC = B // N_CORES          # 64 batches per core
COLS = BC * D              # 4096 (b,d) columns per core
NG = 8                     # PSUM banks
GW = COLS // NG            # 512 columns per bank
MG = 8                     # m-values per chunk group
NMG = M // MG              # 5 m-groups
HB = 128 // MG             # 16 h-values per block
NHB = H // HB              # 8 h-blocks
NCHUNK = NMG * NHB         # 40 contraction chunks

_cache = {}


def _build(reps=1, n_warm=20, n_fill=5, mm_order="chunk", use_pool=False,
           pair=True, pool_noise=False, max_pair=2, direct_store=False,
           full=False, dma2q=False):
    import contextlib

    import concourse.bacc as bacc
    import concourse.mybir as mybir
    from concourse.tile import TileContext

    f32 = mybir.dt.float32
    bf16 = mybir.dt.bfloat16

    nc = bacc.Bacc("TRN2", debug=False, num_devices=N_CORES)

    xkr_d = nc.dram_tensor("xkrep_in", [NHB, 128, COLS], bf16, kind="ExternalInput")
    x0b_d = nc.dram_tensor("x0bc_in", [NMG, 128, COLS], bf16, kind="ExternalInput")
    # pre-transposed: partition-major, 10KB contiguous per partition row
    w3_d = nc.dram_tensor("w3_in", [128, NCHUNK * C], bf16, kind="ExternalInput")
    bias_d = nc.dram_tensor("bias_in", [C, 1], f32, kind="ExternalInput")
    # bias as a single-partition row for the K=1 bias matmul (direct_store)
    biasr_d = nc.dram_tensor("biasr_in", [1, C], bf16, kind="ExternalInput")
    # c-major so each output descriptor is a contiguous 2KB (b,d) run
    out_d = nc.dram_tensor("out", [C, BC, D], f32, kind="ExternalOutput")

    GK = 5
    NGRP = NCHUNK // GK        # 8 groups per pass
    HCOL = COLS // 2           # 2048
    bpg = BC // NG             # 8 batches per bank

    # Per 5-chunk group: one chunk to Pool, remaining four as two adjacent
    # (same-g, j/j+1) DVE pairs.  Chosen so every group pairs cleanly.
    POOL_CHUNKS = (0, 7, 10, 15, 24, 25, 34, 35)
    GROUPS = []
    for gi in range(NGRP):
        ks = list(range(gi * GK, gi * GK + GK))
        pk = [k for k in ks if k in POOL_CHUNKS]
        assert len(pk) == 1
        rest = [k for k in ks if k != pk[0]]
        pairs = [(rest[0], rest[1]), (rest[2], rest[3])]
        for a, b in pairs:
            assert b == a + 1 and a % NHB != NHB - 1 and a // NHB == b // NHB
        GROUPS.append((pk[0], pairs))

    N_DVE_BUF = 5
    N_POOL_BUF = 3

    with TileContext(nc) as tc:
        with (
            tc.tile_pool(name="const", bufs=1) as cpool,
            tc.tile_pool(name="work", bufs=1) as wpool,
            tc.tile_pool(name="outp", bufs=1) as opool,
            tc.tile_pool(name="psum", bufs=1, space="PSUM") as ppool,
        ):
            # ---- SBUF constant tiles ----
            w3_sb = cpool.tile([128, NCHUNK * C], bf16)
            bias_sb = cpool.tile([128, 1], f32)
            biasr_sb = cpool.tile([1, C], bf16, name="biasr")
            ones_sb = cpool.tile([1, GW], bf16, name="ones1")
            xkall = cpool.tile([128, NHB * COLS], bf16, name="xkall")
            xkreps = [xkall[:, i * COLS:(i + 1) * COLS] for i in range(NHB)]
            xk3 = xkall.rearrange("p (j c) -> p j c", c=COLS)
            x0bcs = [
                cpool.tile([128, COLS], bf16, name=f"x0b{i}", tag=f"x0b{i}")
                for i in range(NMG)
            ]

            # ---- prologue DMA: half-column granularity, first-use order ---
            # Single (sync-queue) stream at full DMA bandwidth.  w3 rides in
            # three slices ordered by the chunk ranges that consume them;
            # bias is only needed by the first eviction (~40us in).
            _ldn = [0]

            def ld(kind, i, c0, c1):
                t = xkreps[i] if kind == "x" else x0bcs[i]
                src = (xkr_d if kind == "x" else x0b_d).ap()[i]
                # two DGE queues so two DMA engines stream concurrently
                eng = nc.sync if (not dma2q or _ldn[0] % 2 == 0) else nc.scalar
                _ldn[0] += 1
                eng.dma_start(out=t[:, c0:c1], in_=src[:, c0:c1])

            use_order = [("0", 0), ("x", 0), ("x", 1), ("w", 0), ("x", 2),
                         ("x", 3), ("x", 4), ("x", 5), ("x", 6), ("x", 7),
                         ("0", 1), ("w", 1), ("0", 2), ("w", 2), ("0", 3),
                         ("0", 4)]
            w_slices = [(0, 10 * C), (10 * C, 25 * C), (25 * C, NCHUNK * C)]
            nc.vector.memset(ones_sb, 1.0)
            nc.sync.dma_start(out=biasr_sb, in_=biasr_d.ap())

            def emit_prologue():
                for half, (c0, c1) in enumerate([(0, HCOL), (HCOL, COLS)]):
                    for kind, i in use_order:
                        if kind == "w":
                            if half == 0:
                                s0, s1 = w_slices[i]
                                nc.sync.dma_start(
                                    out=w3_sb[:, s0:s1],
                                    in_=w3_d.ap()[:, s0:s1],
                                )
                        else:
                            ld(kind, i, c0, c1)
                    if half == 0:
                        nc.sync.dma_start(out=bias_sb, in_=bias_d.ap())

            if not full:
                emit_prologue()

            loop_ctx = (
                tc.For_i(
                    0, reps, 1,
                    hint_engines=(mybir.EngineType.PE,),
                    staggered_reset=True,
                )
                if reps > 1
                else contextlib.nullcontext()
            )
            with loop_ctx:
                if full:
                    emit_prologue()
                psums = [
                    ppool.tile([128, GW], f32, name=f"ps{q}", tag=f"ps{q}")
                    for q in range(NG)
                ]

                if reps == 1 or full:
                    # Warm the PE (clock-gate needs ~3.4us of sustained
                    # activity to reach 2.4 GHz) with dummy matmuls while
                    # the first operand DMAs are in flight.  Pass A uses
                    # banks 0-3, so bank 7 absorbs the garbage; pass B's
                    # first real MM into bank 7 resets it via start=True.
                    scratch = cpool.tile([128, GW], bf16)
                    nc.scalar.memzero(scratch)
                    for _ in range(n_warm):
                        nc.tensor.matmul(
                            psums[7], lhsT=scratch[:, :128], rhs=scratch,
                            start=True, stop=True,
                        )

                ndve = 0
                npool = 0
                passes = [(0, HCOL, (0, 1, 2, 3)), (HCOL, COLS, (4, 5, 6, 7))]
                for pi, (c0, c1, banks) in enumerate(passes):
                    width = c1 - c0
                    for gi, (pool_k, pairs) in enumerate(GROUPS):
                        entries = []
                        # Pool chunk first so the slow engine's stream is
                        # maximally early; 3 rotating bufs let it run ahead.
                        if use_pool or pool_noise:
                            g, j = divmod(pool_k, NHB)
                            po = wpool.tile(
                                [128, width], bf16, name=f"po{pi}_{gi}",
                                tag=f"po{npool % N_POOL_BUF}", bufs=1,
                            )
                            npool += 1
                            nc.gpsimd.tensor_mul(
                                po, xkreps[j][:, c0:c1], x0bcs[g][:, c0:c1]
                            )
                        dve_ks = []
                        for ka, kb in pairs:
                            dve_ks.extend([ka, kb])
                        if not use_pool:
                            dve_ks.append(pool_k)
                            dve_ks.sort()
                        if pair:
                            # greedy runs of adjacent same-g chunks, up to
                            # max_pair wide: one DVE op per run with the x0
                            # slice broadcast along the run dim
                            units = []
                            i = 0
                            while i < len(dve_ks):
                                k = dve_ks[i]
                                run = 1
                                while (run < max_pair
                                       and i + run < len(dve_ks)
                                       and dve_ks[i + run] == k + run
                                       and (k + run) % NHB != 0):
                                    run += 1
                                units.append((k, run))
                                i += run
                        else:
                            units = [(k, 1) for k in dve_ks]
                        for k, nun in units:
                            g2, j2 = divmod(k, NHB)
                            nbuf = N_DVE_BUF if nun <= 2 else 3
                            t = wpool.tile(
                                [128, nun * width], bf16,
                                name=f"od{pi}_{gi}_{k}",
                                tag=f"od{nun}_{ndve % nbuf}", bufs=1,
                            )
                            ndve += 1
                            if nun > 1:
                                nc.vector.tensor_mul(
                                    t.rearrange("p (u c) -> p u c", u=nun),
                                    xk3[:, j2:j2 + nun, c0:c1],
                                    x0bcs[g2][:, c0:c1]
                                    .unsqueeze(1)
                                    .broadcast_to([128, nun, width]),
                                )
                                for u in range(nun):
                                    entries.append((k + u, t, u * width))
                            else:
                                nc.vector.tensor_mul(
                                    t, xkreps[j2][:, c0:c1],
                                    x0bcs[g2][:, c0:c1],
                                )
                                entries.append((k, t, 0))
                        # Pool chunk consumed last: it was produced with ~1
                        # group of lag.  Chunk-major MM order (all banks per
                        # chunk) so the PE consumes each chunk the moment it
                        # is produced instead of stalling a whole bank sweep
                        # on the last chunk of the group; also reuses the
                        # stationary w3 chunk across the 4 bank MMs.
                        if use_pool:
                            entries.append((pool_k, po, 0))
                        ne = len(entries)
                        if mm_order == "chunk":
                            order = [(n, qi) for n in range(ne)
                                     for qi in range(len(banks))]
                        else:
                            order = [(n, qi) for qi in range(len(banks))
                                     for n in range(ne)]
                        for n, qi in order:
                            k, t, off = entries[n]
                            qb = banks[qi]
                            nc.tensor.matmul(
                                psums[qb],
                                lhsT=w3_sb[:, k * C:(k + 1) * C],
                                rhs=t[:, off + qi * GW:
                                      off + (qi + 1) * GW],
                                start=(gi == 0 and n == 0),
                                stop=(not direct_store
                                      and gi == NGRP - 1 and n == ne - 1),
                            )
                        if (reps == 1 or full) and pi == 0 and gi < 6:
                            # bootstrap filler: the prologue DMA stream can
                            # momentarily starve the PE here; dep-free
                            # scratch matmuls into the not-yet-active bank 7
                            # absorb the stall (an idle gap resets the clock
                            # ramp, costing ~2x on the next ~3us of MMs)
                            for _ in range(n_fill):
                                nc.tensor.matmul(
                                    psums[7], lhsT=scratch[:, :128],
                                    rhs=scratch, start=True, stop=True,
                                )
                    if direct_store:
                        # bias folded into a K=1 matmul (the stop MM of each
                        # bank), then DMA the PSUM bank straight to DRAM:
                        # no ScalarE eviction, no SBUF store traffic.
                        for qi, qb in enumerate(banks):
                            nc.tensor.matmul(
                                psums[qb],
                                lhsT=biasr_sb[0:1, :],
                                rhs=ones_sb[0:1, :],
                                start=False,
                                stop=True,
                            )
                            nc.scalar.dma_start(
                                out=out_d.ap()[:, qb * bpg:(qb + 1) * bpg, :],
                                in_=psums[qb],
                            )
                    else:
                        # bias-add eviction per bank on ScalarE; stores ride
                        # the ScalarE DGE queue so they never queue behind
                        # loads.
                        for qi, qb in enumerate(banks):
                            out_sb = opool.tile(
                                [128, GW], f32, name=f"osb{pi}{qb}",
                                tag=f"osb{qi}",
                            )
                            nc.scalar.activation(
                                out_sb,
                                psums[qb],
                                mybir.ActivationFunctionType.Identity,
                                bias=bias_sb[:, 0:1],
                                scale=1.0,
                            )
                            nc.scalar.dma_start(
                                out=out_d.ap()[:, qb * bpg:(qb + 1) * bpg, :],
                                in_=out_sb,
                            )

    nc.compile()
    return nc


def _prep_host(x0, xk, W, b):
    """Host-side layout prep (no arithmetic): shard, transpose, replicate."""
    part = np.arange(128)
    hh = (part % HB)[None, :] + HB * np.arange(NHB)[:, None]   # [NHB, 128]
    mm = (part // HB)[None, :] + MG * np.arange(NMG)[:, None]  # [NMG, 128]

    Wr = W.reshape(C, H, M)
    w3 = np.empty((128, NCHUNK, C), ml_dtypes.bfloat16)
    for g in range(NMG):
        for j in range(NHB):
            w3[:, g * NHB + j, :] = Wr[:, hh[j], mm[g]].T.astype(
                ml_dtypes.bfloat16
            )
    w3 = np.ascontiguousarray(w3.reshape(128, NCHUNK * C))
    bias = np.ascontiguousarray(b.reshape(C, 1)).astype(np.float32)

    in_maps = []
    for k in range(N_CORES):
        x0s = x0[k * BC:(k + 1) * BC]            # [BC, M, D]
        xks = xk[k * BC:(k + 1) * BC]            # [BC, H, D]
        xk2 = (
            np.ascontiguousarray(xks.transpose(1, 0, 2))
            .reshape(H, COLS)
            .astype(ml_dtypes.bfloat16)
        )
        x02 = (
            np.ascontiguousarray(x0s.transpose(1, 0, 2))
            .reshape(M, COLS)
            .astype(ml_dtypes.bfloat16)
        )
        in_maps.append(
            {
                "xkrep_in": np.ascontiguousarray(xk2[hh]),
                "x0bc_in": np.ascontiguousarray(x02[mm]),
                "w3_in": w3,
                "bias_in": bias,
                "biasr_in": np.ascontiguousarray(
                    b.reshape(1, C)
                ).astype(ml_dtypes.bfloat16),
            }
        )
    return in_maps


def _run(in_maps, **kwargs):
    from concourse import bass_utils

    if "nc" not in _cache:
        _cache["nc"] = _build()
    return bass_utils.run_bass_kernel_spmd(
        _cache["nc"], in_maps, core_ids=list(range(N_CORES)), **kwargs
    )


def kernel(x0, xk, W, b, _bench=[None]):
    x0 = np.asarray(x0, dtype=np.float32)
    xk = np.asarray(xk, dtype=np.float32)
    W = np.asarray(W, dtype=np.float32)
    b = np.asarray(b, dtype=np.float32)
    in_maps = _prep_host(x0, xk, W, b)
    res = _run(in_maps)
    _bench[0] = res
    # per-core out is c-major [C, BC, D]; restore [BC, C, D] and stack cores
    out = np.concatenate(
        [np.transpose(r["out"], (1, 0, 2)) for r in res.results], axis=0
    )
    return np.ascontiguousarray(out, dtype=np.float32)
